# revision 21
# baseline (speedup 1.0000x reference)
"""Trainium2 Bass kernel for nn_NeuralGeneratedConv (per-pixel generated 3x3 conv).

Contract: kernel(**inputs) takes FULL inputs (as produced by setup_inputs())
and returns the FULL [4, 16, 128, 128] float32 output. Internally the work is
sharded over 8 NeuronCores: core = batch*2 + x_half; each core handles one
batch image and a 64-column slice of the output (all 128 rows).

Per-core device program (pixels live on SBUF partitions as image rows y,
tiles iterate image columns x):
  1. net_inT[2, 8192] = coords - foa  (ACT, cast to fp32r)
  2. hT[256, 8192] = relu(W1.T @ net_inT + b1)  (PE K=2 matmuls + ACT relu)
  3. per column x0: net_out[128y, 2304] = hT_x0.T @ W2 (+ b2) in PSUM (fp32r
     matmuls, fp32 accumulate), where the 2304 axis is (o, i, dy, dx)
  4. apply: out[y, o] = sum_{i,dy,dx} net_out[y, (o,i,dy,dx)] * patch[y, (i,dy,dx)]
     via a custom DVE op computing a running cumsum of products, followed by a
     strided subtract of per-o segment boundaries. patch rows are free-dim
     slices of a y-shifted, reflect-padded image copy (Rall) built once by DMA.
"""
import numpy as np

import concourse.bass as bass
import concourse.tile as tile
from concourse import bacc, mybir
from concourse.bass_utils import run_bass_kernel_spmd

B, CI, CO, H, W, KS = 4, 16, 16, 128, 128, 3
HID = 256
NCORES = 8
XH = W // 2          # 64 columns per core
NPIX = H * XH        # 8192 pixels per core
OIQ = CO * CI * KS * KS  # 2304
PAGE = CI * KS * KS      # 144 elements per output channel
# split the 2304-wide net_out into two PSUM tiles; matmul outputs must not
# cross PSUM bank boundaries (512 fp32), so use 7+9 output channels with
# bank-aligned moving-dim slices that all stay >= 256 wide (fp32r full rate)
HALF_OS = (7, 9)                      # o's per scan half
HALF_FS = (7 * PAGE, 9 * PAGE)        # 1008, 1296 (3 PSUM banks padded)
MM_SLICES = ((512, 496), (512, 512, 272))
XPAD = XH + 2            # 66 columns per core incl. halo (host pre-padded)
YPAD = H + 2             # 130 rows incl. reflect halo (host pre-padded)
NC_PLANES = CI * KS      # 48 (i, dy) planes

_DT = mybir.dt


# --------------------------------------------------------------------------
# custom DVE op: out[p, t] = s0[p] + sum_{u<=t} in0[p, u] * in1[p, u]
# --------------------------------------------------------------------------
def _mul_cumsum_ref(in0, in1, c0, c1, c2):
    P = in0.shape[0]
    a = np.asarray(in0, np.float32).reshape(P, -1)
    b = np.asarray(in1, np.float32).reshape(P, -1)
    seed = (
        np.asarray(c0, np.float32).reshape(-1, 1)
        if isinstance(c0, np.ndarray)
        else np.float32(c0)
    )
    return (seed + np.cumsum(a * b, axis=1, dtype=np.float32)).astype(np.float32)


def _register_mul_cumsum():
    from concourse import dve_ops
    from concourse.dve_spec import Spec, Src0, Src1, C0, AluOp, scan, lower
    from concourse.dve_uop import DveOpSpec

    name = "MUL_CUMSUM_ANT"
    if name in dve_ops._SUB_OPCODE_FOR_NAME:
        return next(op for op in dve_ops.OPS if op.name == name)
    spec = Spec(body=scan(AluOp.ADD, Src0 * Src1, init=C0), reference=_mul_cumsum_ref)
    row = dve_ops._CUSTOM_DVE_ROW_BASE + len(dve_ops.OPS)
    assert row < 0x20, "custom-DVE opcode rows exhausted"
    shas = {}
    for ver in ("v3", "v4"):
        s = DveOpSpec(name=name, opcode=row, uops=lower(spec, ver=ver), rd1_en=True)
        shas[ver] = s.sha(ver)
    op = dve_ops.DveOp(name, spec, subdim=False, uops_sha=shas)
    dve_ops.OPS.append(op)
    dve_ops._SUB_OPCODE_FOR_NAME[name] = row
    dve_ops.CUSTOM_DVE_SPECS[name] = spec
    return op


# --------------------------------------------------------------------------
# device program
# --------------------------------------------------------------------------
def _build(use_b2: bool, mm_dtype: str = "float16"):
    mm_dt = getattr(_DT, mm_dtype)
    op = _register_mul_cumsum()
    nc = bacc.Bacc("TRN2", target_bir_lowering=False, debug=False)

    img = nc.dram_tensor("img", [CI, YPAD, XPAD], _DT.float32, kind="ExternalInput").ap()
    foa = nc.dram_tensor("foa", [2, 1], _DT.float32, kind="ExternalInput").ap()
    coords = nc.dram_tensor("coords", [2, NPIX], _DT.float32, kind="ExternalInput").ap()
    w1 = nc.dram_tensor("w1", [2, HID], mm_dt, kind="ExternalInput").ap()
    b1 = nc.dram_tensor("b1", [HID], _DT.float32, kind="ExternalInput").ap()
    w2 = nc.dram_tensor("w2", [HID, OIQ], mm_dt, kind="ExternalInput").ap()
    b2 = nc.dram_tensor("b2", [1, OIQ], mm_dt, kind="ExternalInput").ap()
    res = nc.dram_tensor("res", [CO, H, XH], _DT.float32, kind="ExternalOutput").ap()

    with tile.TileContext(nc) as tc:
        from contextlib import ExitStack

        ctx = ExitStack()
        with ctx:
            cp = ctx.enter_context(tc.tile_pool(name="const", bufs=1))

            # ---- persistent tiles ----
            w1_sb = [cp.tile([2, 128], mm_dt, tag=f"w1_{c}", name=f"w1sb{c}") for c in range(2)]
            b1_sb = [cp.tile([128, 1], _DT.float32, tag=f"b1_{c}", name=f"b1sb{c}") for c in range(2)]
            w2_sb = [cp.tile([128, OIQ], mm_dt, tag=f"w2_{c}", name=f"w2sb{c}") for c in range(2)]
            coords_sb = cp.tile([2, NPIX], _DT.float32, tag="coords")
            foa_sb = cp.tile([2, 1], _DT.float32, tag="foa")
            nfoa_sb = cp.tile([2, 1], _DT.float32, tag="nfoa")
            rall = cp.tile([128, NC_PLANES * XPAD], _DT.float32, tag="rall")
            ht_sb = [cp.tile([128, NPIX], mm_dt, tag=f"ht_{c}", name=f"htsb{c}") for c in range(2)]
            out_acc = cp.tile([128, CO * XH], _DT.float32, tag="out_acc")
            scratch = cp.tile([128, 16 + OIQ], _DT.float32, tag="scratch")
            if use_b2:
                b2_sb = cp.tile([1, OIQ], mm_dt, tag="b2")
                ones_sb = cp.tile([1, 128], mm_dt, tag="ones")

            # ---- input DMAs (weights pre-cast to fp16 on host) ----
            nc.sync.dma_start(coords_sb[:], coords[:])
            nc.sync.dma_start(foa_sb[:], foa[:])
            for c in range(2):
                nc.sync.dma_start(w1_sb[c][:], w1[:, c * 128:(c + 1) * 128])
                nc.sync.dma_start(b1_sb[c][:], b1[c * 128:(c + 1) * 128].unsqueeze(1))
                nc.sync.dma_start(w2_sb[c][:], w2[c * 128:(c + 1) * 128, :])
            if use_b2:
                nc.sync.dma_start(b2_sb[:], b2[:])
                nc.vector.memset(ones_sb[:], 1.0)

            # ---- Rall: y-shifted image copies (host already reflect-padded) ----
            # layout [y_part, (c, x')] with c = i*3 + dy; img is [i, y+1, x+1]
            rall_v = rall[:].rearrange("p (i d x) -> p i d x", d=KS, x=XPAD)
            for d in range(KS):  # row shift dy = d - 1
                nc.sync.dma_start(
                    rall_v[:, :, d, :], img[:, d:d + H, :].rearrange("i y x -> y i x")
                )
            rall_cx = rall[:].rearrange("p (c x) -> p c x", x=XPAD)

            # ---- zero the seed columns of the scan scratch ----
            nc.vector.memset(scratch[:, 0:16], 0.0)

            # ---- negate foa ----
            nc.vector.tensor_scalar_mul(nfoa_sb[:], foa_sb[:], -1.0)

            # ---- main loop; hT blocks (phase 1) interleaved just-in-time ----
            PB = 512
            COLS_PER_PB = PB // H  # 4
            with tc.tile_pool(name="netin", bufs=3) as nip, \
                 tc.tile_pool(name="hps", bufs=2, space="PSUM") as hps, \
                 tc.tile_pool(name="patch", bufs=2) as pp, \
                 tc.tile_pool(name="mps", bufs=2, space="PSUM") as mps:
                def emit_ht_block(pb):
                    ni = nip.tile([2, PB], mm_dt, tag="ni", name=f"ni{pb}")
                    nc.scalar.add(ni[:], coords_sb[:, pb * PB:(pb + 1) * PB], nfoa_sb[:])
                    for c in range(2):
                        ps = hps.tile([128, PB], _DT.float32, tag="hps", name=f"hps{pb}_{c}")
                        nc.tensor.matmul(ps[:], w1_sb[c][:], ni[:], start=True, stop=True)
                        nc.scalar.activation(
                            ht_sb[c][:, pb * PB:(pb + 1) * PB], ps[:],
                            mybir.ActivationFunctionType.Relu,
                            bias=b1_sb[c][:], scale=1.0,
                        )

                for x0 in range(XH):
                    if x0 % COLS_PER_PB == 0:
                        emit_ht_block(x0 // COLS_PER_PB)
                    # patch rows for this column: [y, (i, dy, dx)] -> dense [128, 144]
                    pt = pp.tile([128, PAGE], _DT.float32, tag="pt")
                    nc.scalar.copy(
                        pt[:].rearrange("p (c x) -> p c x", x=KS),
                        rall_cx[:, :, x0:x0 + KS],
                    )
                    pss = [
                        mps.tile([128, max(HALF_FS)], _DT.float32, tag="mps",
                                 name=f"mps{x0}_{hf}")
                        for hf in range(2)
                    ]
                    # kc-outer: the 5 N-slices of both halves reuse one
                    # stationary hT tile, so only 2 weight swaps per column
                    for c in range(2):
                        hf_base = 0
                        for hf in range(2):
                            off = 0
                            for nw in MM_SLICES[hf]:
                                nc.tensor.matmul(
                                    pss[hf][:, off:off + nw],
                                    ht_sb[c][:, x0 * 128:(x0 + 1) * 128],
                                    w2_sb[c][:, hf_base + off: hf_base + off + nw],
                                    start=(c == 0),
                                    stop=(c == 1 and not use_b2),
                                )
                                off += nw
                            hf_base += HALF_FS[hf]
                    if use_b2:
                        hf_base = 0
                        for hf in range(2):
                            off = 0
                            for nw in MM_SLICES[hf]:
                                nc.tensor.matmul(
                                    pss[hf][:, off:off + nw],
                                    ones_sb[:],
                                    b2_sb[:, hf_base + off: hf_base + off + nw],
                                    start=False, stop=True,
                                )
                                off += nw
                            hf_base += HALF_FS[hf]
                    hf_base = 0
                    for hf in range(2):
                        n_o, half_f = HALF_OS[hf], HALF_FS[hf]
                        pt_b = pt[:].unsqueeze(1).broadcast_to([128, n_o, PAGE])
                        nc.vector._custom_dve(
                            op,
                            out=scratch[:, 16 + hf_base:16 + hf_base + half_f],
                            in0=pss[hf][:, 0:half_f],
                            in1=pt_b,
                            s0=0.0 if hf == 0 else scratch[:, 15 + hf_base:16 + hf_base],
                        )
                        hf_base += half_f
                    # per-o sums = cumsum at page ends minus previous page end
                    ends = scratch[:, 16:16 + OIQ].rearrange(
                        "p (s n) -> p s n", n=PAGE
                    )[:, :, PAGE - 1]
                    starts = scratch[:, 15:15 + OIQ].rearrange(
                        "p (s n) -> p s n", n=PAGE
                    )[:, :, 0]
                    oa = out_acc[:].rearrange("p (o x) -> p o x", x=XH)
                    nc.gpsimd.tensor_tensor(
                        out=oa[:, :, x0], in0=ends, in1=starts,
                        op=mybir.AluOpType.subtract,
                    )

            # ---- output ----
            nc.sync.dma_start(
                res.rearrange("o y x -> y o x"),
                out_acc[:].rearrange("p (o x) -> p o x", x=XH),
            )
    nc.compile()
    return nc


_cache = {}
MM_DTYPE = "float16"


def _get_nc(use_b2: bool):
    key = (use_b2, MM_DTYPE)
    if key not in _cache:
        _cache[key] = _build(use_b2, MM_DTYPE)
    return _cache[key]


def _make_in_maps(input_data, foa_xy, W1, b1, W2, b2):
    input_data = np.ascontiguousarray(input_data, np.float32)
    foa_xy = np.asarray(foa_xy, np.float32)
    W1 = np.ascontiguousarray(W1, np.float16)
    b1 = np.ascontiguousarray(b1, np.float32)
    W2 = np.ascontiguousarray(W2, np.float16)
    b2 = np.ascontiguousarray(b2, np.float16).reshape(1, OIQ)
    # reflect-pad once: [B, CI, H+2, W+2]
    padded = np.pad(input_data, ((0, 0), (0, 0), (1, 1), (1, 1)), mode="reflect")
    in_maps = []
    for core in range(NCORES):
        b, half = divmod(core, 2)
        c0 = half * XH
        img = np.ascontiguousarray(padded[b, :, :, c0:c0 + XPAD])  # [CI, YPAD, XPAD]
        xs = np.repeat(np.arange(c0, c0 + XH, dtype=np.float32), H)
        ys = np.tile(np.arange(H, dtype=np.float32), XH)
        coords = np.stack([xs, ys], axis=0)  # [2, NPIX], pixel p = x_local*128 + y
        in_maps.append(
            dict(
                img=img,
                foa=foa_xy[b].reshape(2, 1),
                coords=np.ascontiguousarray(coords),
                w1=W1,
                b1=b1,
                w2=W2,
                b2=b2,
            )
        )
    return in_maps


def _run(inputs, trace=False, trace_cores=None):
    use_b2 = bool(np.any(np.asarray(inputs["b2"]) != 0))
    nc = _get_nc(use_b2)
    in_maps = _make_in_maps(**inputs)
    r = run_bass_kernel_spmd(
        nc, in_maps, list(range(NCORES)), trace=trace, trace_cores=trace_cores
    )
    out = np.empty((B, CO, H, W), np.float32)
    for core in range(NCORES):
        b, half = divmod(core, 2)
        out[b, :, :, half * XH:(half + 1) * XH] = r.results[core]["res"]
    return out, r


def kernel(**inputs) -> np.ndarray:
    out, _ = _run(inputs)
    return out
